# revision 30
# baseline (speedup 1.0000x reference)
"""LocalAttentionBlock on 8 trn2 cores.

Sharding: 8 cores = 2 batches x 4 sequence blocks of 512 queries.
Each core gets a zero-padded 1024-wide context window (block +/- 256),
transposed on host to [D, s] layout.  All matmuls in bf16 (f32 psum
accumulate) -- f32r runs 2-pass on the PE, bf16 single-pass.

Per-core pipeline (S^T layout: s on partitions, q on free dim):
  One fused [k; v] projection pass (k lanes 0:64, v 64:128); k is
  duplicated to partitions 64:127 by a single SBUF->SBUF DMA so odd
  heads can matmul from base_partition 64.  v re-transposed on PE into
  v_aug tiles whose extra 64 columns hold the per-row validity bit --
  the AV matmul then emits the attention numerator and a 64-way
  replicated softmax denominator in one pass.
  S^T band pieces are trimmed to the exact 128-granular band (2560
  cols/head) and packed gap-free into a 2-bank psum tensor A
  (jt3,jt4) and a 3-bank tensor B (jt2+jt0 | jt1+jt6 | jt5+jt7).
  exp on ACT with the 1/8 scale folded in, bf16 out; the |i-j|<=256
  diagonal is cut by eight 128-wide gpsimd affine_selects per head.
  Normalization: reciprocal_approx_fast on the denom replica lanes,
  SBUF->SBUF DMA shifts it to the attn lanes, one DVE multiply.
  Then y^T = WfT.T @ attn + bf, emitted bf16 and upcast on host.
"""
import sys

import ml_dtypes
import numpy as np

sys.path.insert(0, "/opt/trn_rl_repo")

import concourse.bass as bass  # noqa: E402,F401
import concourse.mybir as mybir  # noqa: E402
import concourse.tile as tile  # noqa: E402
from concourse import bacc  # noqa: E402
from concourse.bass import ts  # noqa: E402
from concourse.bass_utils import run_bass_kernel_spmd  # noqa: E402
from concourse.masks import make_identity  # noqa: E402

F32 = mybir.dt.float32
BF16 = mybir.dt.bfloat16
AF = mybir.ActivationFunctionType
ALU = mybir.AluOpType
BF = ml_dtypes.bfloat16

B, T, D = 2, 2048, 1024
NH, HD = 16, 64
WIN = 256
BLK = 512      # queries per core
CTX = 1024     # padded context width
NCORES = 8

# S^T band pieces (jt, qlo, width, col_offset, start): s-tile jt covers
# s in [128jt, 128jt+128); piece covers q in [qlo, qlo+width).  Trimmed
# to the exact band at 128-col granularity (jt2/jt5 split in two),
# packed gap-free into THREE chunks of 2+2+1 psum banks so the three
# S->exp->S pipelines are short and psO keeps 3 banks; no piece crosses
# a 512-col bank.  start=True on the first piece of each bank (start
# zeroes the whole 2KB bank).
PIECES = {
    "A": [(3, 0, 512, 0, True), (4, 0, 512, 512, True)],
    "B": [(1, 0, 256, 0, True), (6, 256, 256, 256, False),
          (2, 0, 256, 512, True), (2, 256, 128, 768, False),
          (5, 384, 128, 896, False)],
    "C": [(0, 0, 128, 0, True), (7, 384, 128, 128, False),
          (5, 128, 256, 256, False)],
}
EXP_WS = {"A": 1024, "B": 1024, "C": 512}
# Diagonal masks (col_offset, kind), each 128 wide, on gpsimd.
# 'lo' keeps sp - y >= 0; 'hi' keeps -sp + y >= 0 (y = col - offset).
DIAGS = {"A": [(384, "lo"), (512, "hi")],
         "B": [(128, "lo"), (256, "hi"), (768, "lo")],
         "C": [(0, "lo"), (128, "hi"), (256, "hi")]}


def _build():
    nc = bacc.Bacc(None)
    # weights come in pre-swizzled to the SBUF layout so every DMA is a
    # contiguous 2KB-per-partition transfer (strided rearrange DMAs cost
    # ~5us each): wq/wf rows are [p, m|o, dt|ft, c], wkv rows [p, dt, f]
    xT = nc.dram_tensor("xT", [D, CTX], BF16, kind="ExternalInput")
    wqT = nc.dram_tensor("wqT", [128, 8 * D], BF16, kind="ExternalInput")
    wkvT = nc.dram_tensor("wkvT", [128, 8 * 128], BF16, kind="ExternalInput")
    wfT = nc.dram_tensor("wfT", [128, 8 * D], BF16, kind="ExternalInput")
    bfin = nc.dram_tensor("bfin", [D, 1], F32, kind="ExternalInput")
    valid = nc.dram_tensor("valid", [128, 8], F32, kind="ExternalInput")
    yT = nc.dram_tensor("yT", [D, BLK], BF16, kind="ExternalOutput")

    with tile.TileContext(nc) as tc:
        with (
            tc.tile_pool(name="big", bufs=1) as big,
            tc.tile_pool(name="sm", bufs=1) as sm,
            tc.tile_pool(name="recp", bufs=2) as recp,
            tc.tile_pool(name="expp", bufs=2) as expp,
        ):
            # ---- input DMAs (small first; xt before wq before wf) ----
            xt = big.tile([128, 8, CTX], BF16, tag="xt")
            wkv = big.tile([128, 8, 128], BF16, tag="wkv")
            wq = big.tile([128, 8, 8, 128], BF16, tag="wq")
            wf = big.tile([128, 8, 8, 128], BF16, tag="wf")
            nc.sync.dma_start(out=wkv[:, 0, :], in_=wkvT[:, 0:128])
            nc.sync.dma_start(out=xt[:, 0, :], in_=xT[ts(0, 128), :])
            nc.sync.dma_start(out=wkv[:, 1:8, :], in_=wkvT[:, 128:1024])
            for dt in range(1, 8):
                nc.sync.dma_start(out=xt[:, dt, :], in_=xT[ts(dt, 128), :])
            # wq per output-m-tile: q(m) -- and with it the first S heads
            # -- can start as soon as its slice lands
            for m in range(8):
                nc.sync.dma_start(out=wq[:, m, :, :],
                                  in_=wqT[:, ts(m, 1024)])
            bf_sb = sm.tile([128, 8], F32, tag="bf")
            nc.sync.dma_start(
                out=bf_sb, in_=bfin.rearrange("(o p) x -> p (o x)", p=128))
            valid_sb = sm.tile([128, 8], F32, tag="valid")
            nc.sync.dma_start(out=valid_sb, in_=valid[:, :])
            for o in range(0, 8, 4):
                nc.sync.dma_start(out=wf[:, o:o + 4, :, :],
                                  in_=wfT[:, o * 1024:(o + 4) * 1024])
            ident = sm.tile([128, 128], BF16, tag="ident")
            make_identity(nc, ident)
            ones64 = sm.tile([128, 64], BF16, tag="ones64")
            nc.vector.memset(ones64, 1.0)

            # kv_sb: rows 0:64 = k^T, rows 64:128 = v^T; khi rows 64:128
            # carry the same k^T so odd heads matmul from base 64.
            kv_sb = big.tile([128, CTX], BF16, tag="kv")
            khi = big.tile([128, CTX], BF16, tag="khi")
            vaug_e = big.tile([128, 8, 128], BF16, tag="vaug_e")
            vaug_o = big.tile([128, 8, 128], BF16, tag="vaug_o")
            qT = big.tile([128, 8, BLK], BF16, tag="qT")
            # attn split in two so the Wf contraction can start as soon
            # as heads 0-7 have normalized (overlapping the last heads)
            anrm1 = big.tile([128, 4, BLK], BF16, tag="anrm1")
            anrm2 = big.tile([128, 4, BLK], BF16, tag="anrm2")

            # ---- projections (own psum scope, double-buffered) ----
            with tc.tile_pool(name="psproj", bufs=2, space="PSUM") as psp:
                # dt-outer so each xt tile is consumed as it arrives;
                # q(m=0) rides along so the PE is never idle while the
                # xt DMA stream lands
                kv_ps = [psp.tile([128, 512], F32, tag=f"kv{ch}", bufs=1,
                                  name=f"kv_ps{ch}") for ch in range(2)]
                q_ps0 = psp.tile([128, 512], F32, tag="acc", bufs=3)
                for dt in range(8):
                    for ch in range(2):
                        nc.tensor.matmul(kv_ps[ch], wkv[:, dt, :],
                                         xt[:, dt, ts(ch, 512)],
                                         start=(dt == 0), stop=(dt == 7))
                    nc.tensor.matmul(q_ps0, wq[:, 0, dt, :],
                                     xt[:, dt, 256:768],
                                     start=(dt == 0), stop=(dt == 7))
                for ch in range(2):
                    nc.scalar.activation(out=kv_sb[:, ts(ch, 512)],
                                         in_=kv_ps[ch], func=AF.Copy)
                nc.sync.dma_start(out=khi[64:128, :], in_=kv_sb[0:64, :])
                nc.vector.tensor_copy(qT[:, 0, :], q_ps0)

                def emit_vaug():
                    # v_aug: [v | valid*64] (even heads) / [valid*64 | v] (odd)
                    for jt in range(8):
                        t_ps = psp.tile([128, 64], BF16, tag="tp")
                        nc.tensor.transpose(t_ps,
                                            kv_sb[64:128, ts(jt, 128)],
                                            ident[64:128, 64:128])
                        nc.vector.tensor_copy(vaug_e[:, jt, 0:64], t_ps)
                        nc.vector.tensor_copy(vaug_o[:, jt, 64:128], t_ps)
                        nc.vector.tensor_scalar_mul(vaug_e[:, jt, 64:128],
                                                    ones64,
                                                    valid_sb[:, jt:jt + 1])
                        nc.vector.tensor_scalar_mul(vaug_o[:, jt, 0:64],
                                                    ones64,
                                                    valid_sb[:, jt:jt + 1])

                for m in range(1, 8):
                    q_ps = psp.tile([128, 512], F32, tag="acc", bufs=3)
                    for dt in range(8):
                        nc.tensor.matmul(q_ps, wq[:, m, dt, :],
                                         xt[:, dt, 256:768],
                                         start=(dt == 0), stop=(dt == 7))
                    nc.vector.tensor_copy(qT[:, m, :], q_ps)
                    if m == 1:
                        emit_vaug()

            # ---- attention middle (2+2+1+3 psum banks) ----
            with (
                tc.tile_pool(name="psSA", bufs=1, space="PSUM") as psA,
                tc.tile_pool(name="psSB", bufs=1, space="PSUM") as psB,
                tc.tile_pool(name="psSC", bufs=1, space="PSUM") as psC,
                tc.tile_pool(name="psO", bufs=3, space="PSUM") as psO,
            ):
                npc = sum(len(p) for p in PIECES.values())

                def emit_av_norm(h, halves, va):
                    m, r0 = h // 2, 64 * (h % 2)
                    odd = h % 2 == 1
                    o_ps = psO.tile([128, 512], F32, tag="O")
                    nav = 0
                    for (pieces, ex) in halves:
                        for (jt, qlo, w, off, _) in pieces:
                            nc.tensor.matmul(o_ps[:, qlo:qlo + w],
                                             va[:, jt, :],
                                             ex[:, off:off + w],
                                             start=(nav == 0),
                                             stop=(nav == npc - 1),
                                             skip_group_check=True)
                            nav += 1
                    # normalize: denom replicated on the opposite 64
                    # lanes.  reciprocal_approx_fast only works on SBUF
                    # input at partitions 0:64 on HW, so: DVE-copy the
                    # denom out of psum (lane-locked), route via DMA so
                    # the recip runs on the lower lanes, multiply on the
                    # attn lanes.
                    dlo = 0 if odd else 64
                    den = recp.tile([128, BLK], F32, tag="den")
                    rec = recp.tile([128, BLK], F32, tag="rec")
                    nc.vector.tensor_copy(den[dlo:dlo + 64, :],
                                          o_ps[dlo:dlo + 64, :])
                    if odd:
                        # den already on 0:64: recip there, shift rec up
                        nc.vector.reciprocal_approx_fast(
                            rec[0:64, :], den[0:64, :])
                        nc.sync.dma_start(out=rec[64:128, :],
                                          in_=rec[0:64, :])
                    else:
                        # den on 64:128: shift down, recip on 0:64
                        nc.sync.dma_start(out=den[0:64, :],
                                          in_=den[64:128, :])
                        nc.vector.reciprocal_approx_fast(
                            rec[0:64, :], den[0:64, :])
                    anrm = anrm1 if m < 4 else anrm2
                    nc.vector.tensor_mul(anrm[r0:r0 + 64, m % 4, :],
                                         o_ps[r0:r0 + 64, :],
                                         rec[r0:r0 + 64, :])

                pending = None
                for h in range(NH + 1):
                    if h < NH:
                        m, r0 = h // 2, 64 * (h % 2)
                        odd = h % 2 == 1
                        qTh = qT[r0:r0 + 64, m, :]
                        kTh = (khi if odd else kv_sb)[r0:r0 + 64, :]
                        va = vaug_o[:, :, :] if odd else vaug_e[:, :, :]
                        halves = []
                        for (pool, ck) in ((psA, "A"), (psB, "B"),
                                           (psC, "C")):
                            pieces = PIECES[ck]
                            xw = EXP_WS[ck]
                            s_ps = pool.tile([128, xw], F32, tag=f"S{ck}",
                                             name=f"sps{ck}{h}")
                            for (jt, qlo, w, off, first) in pieces:
                                nc.tensor.matmul(s_ps[:, off:off + w],
                                                 kTh[:, ts(jt, 128)],
                                                 qTh[:, qlo:qlo + w],
                                                 start=first, stop=True,
                                                 skip_group_check=True)
                            ex = expp.tile([128, xw], BF16, tag=f"ex{ck}")
                            nc.scalar.activation(out=ex, in_=s_ps,
                                                 func=AF.Exp, scale=0.125)
                            for (doff, kind) in DIAGS[ck]:
                                lo = kind == "lo"
                                nc.gpsimd.affine_select(
                                    out=ex[:, doff:doff + 128],
                                    in_=ex[:, doff:doff + 128],
                                    compare_op=ALU.is_ge,
                                    fill=0.0, base=0,
                                    pattern=[[-1 if lo else 1, 128]],
                                    channel_multiplier=1 if lo else -1)
                            halves.append((pieces, ex))
                        cur = (h, halves, va)
                    else:
                        cur = None
                    if pending is not None:
                        emit_av_norm(*pending)
                    pending = cur

                # ---- y^T = wf.T @ attn_norm + bf (alternates the O
                # psum slots; no pool-close barrier before Wf) ----
                for o in range(8):
                    y_ps = psO.tile([128, 512], F32, tag="O",
                                    name=f"y_ps{o}")
                    for ft in range(8):
                        src = anrm1 if ft < 4 else anrm2
                        nc.tensor.matmul(y_ps, wf[:, o, ft, :],
                                         src[:, ft % 4, :],
                                         start=(ft == 0), stop=(ft == 7))
                    y_sb = big.tile([128, BLK], BF16, tag=f"y{o % 2}",
                                    name=f"y_sb{o}")
                    nc.vector.tensor_scalar_add(y_sb, y_ps,
                                                bf_sb[:, o:o + 1])
                    nc.sync.dma_start(out=yT[ts(o, 128), :], in_=y_sb)

    nc.compile()
    return nc


_NC = None


def _get_nc():
    global _NC
    if _NC is None:
        _NC = _build()
    return _NC


def _swiz(wT):
    """[D, 8blk*128c] row-major -> [p, blk, dt, c] swizzled rows."""
    return np.ascontiguousarray(
        wT.reshape(8, 128, 8, 128).transpose(1, 2, 0, 3).reshape(128, -1))


def _prep_inputs(x, Wq, Wk, Wv, Wf, bf):
    x = np.asarray(x, np.float32)
    wkvT = np.concatenate([np.asarray(Wk, np.float32),
                           np.asarray(Wv, np.float32)], axis=0).T
    shared = {
        # wq/wf: [dt*128+p, m*128+c] -> [p, m*1024 + dt*128 + c]
        "wqT": _swiz(np.asarray(Wq, np.float32).T).astype(BF),
        "wfT": _swiz(np.asarray(Wf, np.float32).T).astype(BF),
        # wkv: [dt*128+p, f] -> [p, dt*128 + f]
        "wkvT": np.ascontiguousarray(
            wkvT.reshape(8, 128, 128).transpose(1, 0, 2).reshape(128, -1)
        ).astype(BF),
        "bfin": np.asarray(bf, np.float32).reshape(D, 1),
    }
    in_maps = []
    for c in range(NCORES):
        b, i = divmod(c, 4)
        g0 = 512 * i - WIN  # global position of ctx col 0
        xTc = np.zeros((D, CTX), np.float32)
        lo, hi = max(0, g0), min(T, g0 + CTX)
        xTc[:, lo - g0:hi - g0] = x[b, lo:hi, :].T
        s = np.arange(CTX)
        vmask = ((s + g0 >= 0) & (s + g0 < T)).astype(np.float32)
        in_maps.append({
            "xT": xTc.astype(BF),
            "valid": np.ascontiguousarray(vmask.reshape(8, 128).T),
            **shared,
        })
    return in_maps


def _run(inputs, trace=False):
    nc = _get_nc()
    in_maps = _prep_inputs(**inputs)
    res = run_bass_kernel_spmd(nc, in_maps, core_ids=list(range(NCORES)),
                               trace=trace)
    x = inputs["x"]
    out = np.empty((B, T, D), np.float32)
    for c in range(NCORES):
        b, i = divmod(c, 4)
        out[b, 512 * i:512 * (i + 1), :] = \
            res.results[c]["yT"].astype(np.float32).T
    return out.astype(np.asarray(x).dtype), res


def kernel(**inputs):
    out, _ = _run(inputs)
    return out


# revision 38
# speedup vs baseline: 1.0143x; 1.0143x over previous
"""LocalAttentionBlock on 8 trn2 cores.

Sharding: 8 cores = 2 batches x 4 sequence blocks of 512 queries.
Each core gets a zero-padded 1024-wide context window (block +/- 256),
transposed on host to [D, s] layout.  All matmuls in bf16 (f32 psum
accumulate) -- f32r runs 2-pass on the PE, bf16 single-pass.

Per-core pipeline (S^T layout: s on partitions, q on free dim):
  One fused [k; v] projection pass (k lanes 0:64, v 64:128); k is
  duplicated to partitions 64:127 by a single SBUF->SBUF DMA so odd
  heads can matmul from base_partition 64.  v re-transposed on PE into
  v_aug tiles whose extra 64 columns hold the per-row validity bit --
  the AV matmul then emits the attention numerator and a 64-way
  replicated softmax denominator in one pass.
  S^T band pieces are trimmed to the exact 128-granular band (2560
  cols/head) and packed gap-free into a 2-bank psum tensor A
  (jt3,jt4) and a 3-bank tensor B (jt2+jt0 | jt1+jt6 | jt5+jt7).
  exp on ACT with the 1/8 scale folded in, bf16 out; the |i-j|<=256
  diagonal is cut by eight 128-wide gpsimd affine_selects per head.
  Normalization: reciprocal_approx_fast on the denom replica lanes,
  SBUF->SBUF DMA shifts it to the attn lanes, one DVE multiply.
  Then y^T = WfT.T @ attn + bf, emitted bf16 and upcast on host.
"""
import sys

import ml_dtypes
import numpy as np

sys.path.insert(0, "/opt/trn_rl_repo")

import concourse.bass as bass  # noqa: E402,F401
import concourse.mybir as mybir  # noqa: E402
import concourse.tile as tile  # noqa: E402
from concourse import bacc  # noqa: E402
from concourse.bass import ts  # noqa: E402
from concourse.bass_utils import run_bass_kernel_spmd  # noqa: E402
from concourse.masks import make_identity  # noqa: E402

F32 = mybir.dt.float32
BF16 = mybir.dt.bfloat16
AF = mybir.ActivationFunctionType
ALU = mybir.AluOpType
BF = ml_dtypes.bfloat16

B, T, D = 2, 2048, 1024
NH, HD = 16, 64
WIN = 256
BLK = 512      # queries per core
CTX = 1024     # padded context width
NCORES = 8

# S^T band pieces (jt, qlo, width, col_offset, start): s-tile jt covers
# s in [128jt, 128jt+128); piece covers q in [qlo, qlo+width).  Trimmed
# to the exact band at 128-col granularity (jt2/jt5 split in two),
# packed gap-free into THREE chunks of 2+2+1 psum banks so the three
# S->exp->S pipelines are short and psO keeps 3 banks; no piece crosses
# a 512-col bank.  start=True on the first piece of each bank (start
# zeroes the whole 2KB bank).
PIECES = {
    "A": [(3, 0, 512, 0, True), (4, 0, 512, 512, True)],
    "B": [(1, 0, 256, 0, True), (6, 256, 256, 256, False),
          (2, 0, 256, 512, True), (2, 256, 128, 768, False),
          (5, 384, 128, 896, False)],
    "C": [(0, 0, 128, 0, True), (7, 384, 128, 128, False),
          (5, 128, 256, 256, False)],
}
EXP_WS = {"A": 1024, "B": 1024, "C": 512}
# Diagonal masks (col_offset, kind), each 128 wide, on gpsimd.
# 'lo' keeps sp - y >= 0; 'hi' keeps -sp + y >= 0 (y = col - offset).
DIAGS = {"A": [(384, "lo"), (512, "hi")],
         "B": [(128, "lo"), (256, "hi"), (768, "lo")],
         "C": [(0, "lo"), (128, "hi"), (256, "hi")]}


def _build():
    nc = bacc.Bacc(None)
    # weights come in pre-swizzled to the SBUF layout so every DMA is a
    # contiguous 2KB-per-partition transfer (strided rearrange DMAs cost
    # ~5us each): wq/wf rows are [p, m|o, dt|ft, c], wkv rows [p, dt, f]
    xT = nc.dram_tensor("xT", [D, CTX], BF16, kind="ExternalInput")
    wqT = nc.dram_tensor("wqT", [128, 8 * D], BF16, kind="ExternalInput")
    wkvT = nc.dram_tensor("wkvT", [128, 8 * 128], BF16, kind="ExternalInput")
    wfT = nc.dram_tensor("wfT", [128, 8 * D], BF16, kind="ExternalInput")
    bfin = nc.dram_tensor("bfin", [D, 1], F32, kind="ExternalInput")
    valid = nc.dram_tensor("valid", [128, 8], F32, kind="ExternalInput")
    yT = nc.dram_tensor("yT", [D, BLK], BF16, kind="ExternalOutput")

    with tile.TileContext(nc) as tc:
        with (
            tc.tile_pool(name="big", bufs=1) as big,
            tc.tile_pool(name="sm", bufs=1) as sm,
            tc.tile_pool(name="recp", bufs=2) as recp,
            tc.tile_pool(name="expp", bufs=3) as expp,
        ):
            # ---- input DMAs (small first; xt before wq before wf) ----
            xt = big.tile([128, 8, CTX], BF16, tag="xt")
            wkv = big.tile([128, 8, 128], BF16, tag="wkv")
            wq = big.tile([128, 8, 8, 128], BF16, tag="wq")
            wf = big.tile([128, 8, 8, 128], BF16, tag="wf")
            nc.sync.dma_start(out=wkv[:, 0, :], in_=wkvT[:, 0:128])
            nc.sync.dma_start(out=xt[:, 0, :], in_=xT[ts(0, 128), :])
            nc.sync.dma_start(out=wkv[:, 1:8, :], in_=wkvT[:, 128:1024])
            for dt in range(1, 8):
                nc.sync.dma_start(out=xt[:, dt, :], in_=xT[ts(dt, 128), :])
            # wq per output-m-tile: q(m) -- and with it the first S heads
            # -- can start as soon as its slice lands
            for m in range(8):
                nc.sync.dma_start(out=wq[:, m, :, :],
                                  in_=wqT[:, ts(m, 1024)])
            bf_sb = sm.tile([128, 8], F32, tag="bf")
            nc.sync.dma_start(
                out=bf_sb, in_=bfin.rearrange("(o p) x -> p (o x)", p=128))
            valid_sb = sm.tile([128, 8], F32, tag="valid")
            nc.sync.dma_start(out=valid_sb, in_=valid[:, :])
            for o in range(0, 8, 4):
                nc.sync.dma_start(out=wf[:, o:o + 4, :, :],
                                  in_=wfT[:, o * 1024:(o + 4) * 1024])
            ident = sm.tile([128, 128], BF16, tag="ident")
            make_identity(nc, ident)
            ones64 = sm.tile([128, 64], BF16, tag="ones64")
            nc.vector.memset(ones64, 1.0)

            # kv_sb: rows 0:64 = k^T, rows 64:128 = v^T; khi rows 64:128
            # carry the same k^T so odd heads matmul from base 64.
            kv_sb = big.tile([128, CTX], BF16, tag="kv")
            khi = big.tile([128, CTX], BF16, tag="khi")
            vaug_e = big.tile([128, 8, 128], BF16, tag="vaug_e")
            vaug_o = big.tile([128, 8, 128], BF16, tag="vaug_o")
            # per-m q tiles so head 2m's S only waits q(m), not q(7)
            qT = [big.tile([128, BLK], BF16, tag=f"qT{m}",
                           name=f"qT{m}") for m in range(8)]
            anrm = big.tile([128, 8, BLK], BF16, tag="anrm")

            # ---- projections (own psum scope, double-buffered) ----
            with tc.tile_pool(name="psproj", bufs=2, space="PSUM") as psp:
                # dt-outer so each xt tile is consumed as it arrives;
                # q(m=0) rides along so the PE is never idle while the
                # xt DMA stream lands
                kv_ps = [psp.tile([128, 512], F32, tag=f"kv{ch}", bufs=1,
                                  name=f"kv_ps{ch}") for ch in range(2)]
                for dt in range(8):
                    for ch in range(2):
                        nc.tensor.matmul(kv_ps[ch], wkv[:, dt, :],
                                         xt[:, dt, ts(ch, 512)],
                                         start=(dt == 0), stop=(dt == 7))
                for ch in range(2):
                    nc.scalar.activation(out=kv_sb[:, ts(ch, 512)],
                                         in_=kv_ps[ch], func=AF.Copy)
                nc.sync.dma_start(out=khi[64:128, :], in_=kv_sb[0:64, :])

                def emit_vaug():
                    # v_aug: [v | valid*64] (even heads) / [valid*64 | v] (odd)
                    for jt in range(8):
                        t_ps = psp.tile([128, 64], BF16, tag="tp")
                        nc.tensor.transpose(t_ps,
                                            kv_sb[64:128, ts(jt, 128)],
                                            ident[64:128, 64:128])
                        nc.vector.tensor_copy(vaug_e[:, jt, 0:64], t_ps)
                        nc.vector.tensor_copy(vaug_o[:, jt, 64:128], t_ps)
                        nc.vector.tensor_scalar_mul(vaug_e[:, jt, 64:128],
                                                    ones64,
                                                    valid_sb[:, jt:jt + 1])
                        nc.vector.tensor_scalar_mul(vaug_o[:, jt, 0:64],
                                                    ones64,
                                                    valid_sb[:, jt:jt + 1])

                for m in range(8):
                    q_ps = psp.tile([128, 512], F32, tag="acc", bufs=3)
                    for dt in range(8):
                        nc.tensor.matmul(q_ps, wq[:, m, dt, :],
                                         xt[:, dt, 256:768],
                                         start=(dt == 0), stop=(dt == 7))
                    nc.vector.tensor_copy(qT[m], q_ps)
                    if m == 0:
                        emit_vaug()

            # ---- attention middle (2+2+1+3 psum banks) ----
            # pool order maps chunks onto banks by when the projection
            # phase frees them: psA on the kv banks (freed first), psO
            # on the q-accumulator banks (AV starts last), psB on the
            # transpose banks, psC on bank 7 (never used by proj) --
            # so heads 0-2's S/exp/mask work overlaps the q chain.
            with (
                tc.tile_pool(name="psSA", bufs=1, space="PSUM") as psA,
                tc.tile_pool(name="psO", bufs=3, space="PSUM") as psO,
                tc.tile_pool(name="psSB", bufs=1, space="PSUM") as psB,
                tc.tile_pool(name="psSC", bufs=1, space="PSUM") as psC,
            ):
                npc = sum(len(p) for p in PIECES.values())

                def emit_av_norm(h, halves, va):
                    m, r0 = h // 2, 64 * (h % 2)
                    odd = h % 2 == 1
                    o_ps = psO.tile([128, 512], F32, tag="O")
                    nav = 0
                    for (pieces, ex) in halves:
                        for (jt, qlo, w, off, _) in pieces:
                            nc.tensor.matmul(o_ps[:, qlo:qlo + w],
                                             va[:, jt, :],
                                             ex[:, off:off + w],
                                             start=(nav == 0),
                                             stop=(nav == npc - 1),
                                             skip_group_check=True)
                            nav += 1
                    # normalize: denom replicated on the opposite 64
                    # lanes.  reciprocal_approx_fast only works on SBUF
                    # input at partitions 0:64 on HW, so: DVE-copy the
                    # denom out of psum (lane-locked), route via DMA so
                    # the recip runs on the lower lanes, multiply on the
                    # attn lanes.
                    dlo = 0 if odd else 64
                    den = recp.tile([128, BLK], F32, tag="den")
                    rec = recp.tile([128, BLK], F32, tag="rec")
                    nc.vector.tensor_copy(den[dlo:dlo + 64, :],
                                          o_ps[dlo:dlo + 64, :])
                    if odd:
                        # den already on 0:64: recip there, shift rec up
                        nc.vector.reciprocal_approx_fast(
                            rec[0:64, :], den[0:64, :])
                        nc.sync.dma_start(out=rec[64:128, :],
                                          in_=rec[0:64, :])
                    else:
                        # den on 64:128: shift down, recip on 0:64
                        nc.sync.dma_start(out=den[0:64, :],
                                          in_=den[64:128, :])
                        nc.vector.reciprocal_approx_fast(
                            rec[0:64, :], den[0:64, :])
                    nc.vector.tensor_mul(anrm[r0:r0 + 64, m, :],
                                         o_ps[r0:r0 + 64, :],
                                         rec[r0:r0 + 64, :])

                pending = None
                for h in range(NH + 1):
                    if h < NH:
                        m, r0 = h // 2, 64 * (h % 2)
                        odd = h % 2 == 1
                        qTh = qT[m][r0:r0 + 64, :]
                        kTh = (khi if odd else kv_sb)[r0:r0 + 64, :]
                        va = vaug_o[:, :, :] if odd else vaug_e[:, :, :]
                        halves = []
                        for (pool, ck) in ((psA, "A"), (psB, "B"),
                                           (psC, "C")):
                            pieces = PIECES[ck]
                            xw = EXP_WS[ck]
                            s_ps = pool.tile([128, xw], F32, tag=f"S{ck}",
                                             name=f"sps{ck}{h}")
                            for (jt, qlo, w, off, first) in pieces:
                                nc.tensor.matmul(s_ps[:, off:off + w],
                                                 kTh[:, ts(jt, 128)],
                                                 qTh[:, qlo:qlo + w],
                                                 start=first, stop=True,
                                                 skip_group_check=True)
                            ex = expp.tile([128, xw], BF16, tag=f"ex{ck}")
                            nc.scalar.activation(out=ex, in_=s_ps,
                                                 func=AF.Exp, scale=0.125)
                            for (doff, kind) in DIAGS[ck]:
                                lo = kind == "lo"
                                nc.gpsimd.affine_select(
                                    out=ex[:, doff:doff + 128],
                                    in_=ex[:, doff:doff + 128],
                                    compare_op=ALU.is_ge,
                                    fill=0.0, base=0,
                                    pattern=[[-1 if lo else 1, 128]],
                                    channel_multiplier=1 if lo else -1)
                            halves.append((pieces, ex))
                        cur = (h, halves, va)
                    else:
                        cur = None
                    if pending is not None:
                        emit_av_norm(*pending)
                    pending = cur

                # ---- y^T = wf.T @ attn_norm + bf (alternates the O
                # psum slots; no pool-close barrier before Wf) ----
                for o in range(8):
                    y_ps = psO.tile([128, 512], F32, tag="O",
                                    name=f"y_ps{o}")
                    for ft in range(8):
                        nc.tensor.matmul(y_ps, wf[:, o, ft, :],
                                         anrm[:, ft, :],
                                         start=(ft == 0), stop=(ft == 7))
                    y_sb = big.tile([128, BLK], BF16, tag=f"y{o % 2}",
                                    name=f"y_sb{o}")
                    nc.vector.tensor_scalar_add(y_sb, y_ps,
                                                bf_sb[:, o:o + 1])
                    nc.sync.dma_start(out=yT[ts(o, 128), :], in_=y_sb)

    nc.compile()
    return nc


_NC = None


def _get_nc():
    global _NC
    if _NC is None:
        _NC = _build()
    return _NC


def _swiz(wT):
    """[D, 8blk*128c] row-major -> [p, blk, dt, c] swizzled rows."""
    return np.ascontiguousarray(
        wT.reshape(8, 128, 8, 128).transpose(1, 2, 0, 3).reshape(128, -1))


def _prep_inputs(x, Wq, Wk, Wv, Wf, bf):
    x = np.asarray(x, np.float32)
    wkvT = np.concatenate([np.asarray(Wk, np.float32),
                           np.asarray(Wv, np.float32)], axis=0).T
    shared = {
        # wq/wf: [dt*128+p, m*128+c] -> [p, m*1024 + dt*128 + c]
        "wqT": _swiz(np.asarray(Wq, np.float32).T).astype(BF),
        "wfT": _swiz(np.asarray(Wf, np.float32).T).astype(BF),
        # wkv: [dt*128+p, f] -> [p, dt*128 + f]
        "wkvT": np.ascontiguousarray(
            wkvT.reshape(8, 128, 128).transpose(1, 0, 2).reshape(128, -1)
        ).astype(BF),
        "bfin": np.asarray(bf, np.float32).reshape(D, 1),
    }
    in_maps = []
    for c in range(NCORES):
        b, i = divmod(c, 4)
        g0 = 512 * i - WIN  # global position of ctx col 0
        xTc = np.zeros((D, CTX), np.float32)
        lo, hi = max(0, g0), min(T, g0 + CTX)
        xTc[:, lo - g0:hi - g0] = x[b, lo:hi, :].T
        s = np.arange(CTX)
        vmask = ((s + g0 >= 0) & (s + g0 < T)).astype(np.float32)
        in_maps.append({
            "xT": xTc.astype(BF),
            "valid": np.ascontiguousarray(vmask.reshape(8, 128).T),
            **shared,
        })
    return in_maps


def _run(inputs, trace=False):
    nc = _get_nc()
    in_maps = _prep_inputs(**inputs)
    res = run_bass_kernel_spmd(nc, in_maps, core_ids=list(range(NCORES)),
                               trace=trace)
    x = inputs["x"]
    out = np.empty((B, T, D), np.float32)
    for c in range(NCORES):
        b, i = divmod(c, 4)
        out[b, 512 * i:512 * (i + 1), :] = \
            res.results[c]["yT"].astype(np.float32).T
    return out.astype(np.asarray(x).dtype), res


def kernel(**inputs):
    out, _ = _run(inputs)
    return out
